# revision 22
# baseline (speedup 1.0000x reference)
"""Trainium2 Bass kernel for nn_DiffPool_48696339202114.

Math notes (derived from the reference program, valid for ANY input values):
  * proto_k is k identical rows of `proto`, so sim = a @ proto_k.T has k
    identical columns -> softmax over axis=1 is EXACTLY uniform 1/k
    (exp(0)=1, k=4096 a power of two, so A = 1/4096 exactly in fp32).
  * argmax(A, axis=1) == 0 for every row  -> cluster == 0 everywhere
    -> new_src == new_dst == 0 -> edge_mask all False
    -> new_edge_attr == zeros, new_edge_index == zeros.
  * node_types is all zeros -> new_node_types = A.T @ 0 = zeros.
  * new_x = A.T @ h = broadcast of (1/k) * sum_n h[n] over k rows.
  So the only data-dependent compute is h = LayerNorm(x + segsum(msg)) and
  its column sum.  The device computes the per-edge message pipeline
  (edge-MLP "te", multiply with gathered node features, segment-sum,
  LayerNorm, column-sum); the host does index prep, the tiny per-node MLP
  (N=8192 rows, ~0.7% of total FLOPs), and assembles constant outputs.

Sharding: edges sorted by destination node; core c owns destination nodes
[1024c, 1024c+1024) and all their incoming edges.  Node features replicated.
No collective needed: each core returns a partial [1,32] column-sum which
the host adds.
"""

import os
import numpy as np
import ml_dtypes

import concourse.bass as bass
import concourse.bacc as bacc
import concourse.tile as tile
from concourse import mybir
from concourse.bass_utils import run_bass_kernel_spmd
from concourse.masks import make_identity

# ---------------- problem constants (hardcoded per contract) ----------------
N = 8192
E = 524288
ND = 32
ED = 16
K_CLUST = 4096
EPS = 1e-5
NCORES = 8
NPC = N // NCORES          # nodes per core = 1024
NWIN = NPC // 128          # 128-node windows per core = 8
SLOT = 16                  # edges per slot (fixed-size level-1 reduce)
TPW = 5                    # slot-tiles (128 slots) per window
SPW = TPW * 128            # slots per window = 640
CAP_W = SPW * SLOT         # edge capacity per window = 10240
S_CORE = NWIN * SPW        # slots per core = 5120
EP = S_CORE * SLOT         # padded edge positions per core = 81920
CQ = EP // 4               # 4-packed columns = 20480
CHUNK = 512
NCHUNK = CQ // CHUNK       # 40
NT = NWIN * TPW            # slot-tiles per core = 40

F32 = mybir.dt.float32
BF16 = mybir.dt.bfloat16
BF = ml_dtypes.bfloat16

LAST_EXEC_NS = None
LAST_RESULTS = None
_PROG_CACHE = {}


def _lrelu_np(v):
    return np.where(v >= 0, v, np.float32(0.1) * v).astype(np.float32)


# --------------------------- bass program ----------------------------------
def _build_program():
    nc = bacc.Bacc()
    ea = nc.declare_dram_parameter("ea4p", [68, CQ], BF16, isOutput=False)
    xt = nc.declare_dram_parameter("xt4p", [128, CQ], BF16, isOutput=False)
    oh = nc.declare_dram_parameter("onehot", [128, NT * 128], BF16, isOutput=False)
    w1 = nc.declare_dram_parameter("w1p", [68, 128], BF16, isOutput=False)
    w2 = nc.declare_dram_parameter("w2p", [128, 128], BF16, isOutput=False)
    w2b = nc.declare_dram_parameter("w2bp", [68, 128], BF16, isOutput=False)
    b2 = nc.declare_dram_parameter("b2p", [128, 1], F32, isOutput=False)
    xw = nc.declare_dram_parameter("xw", [128, NWIN * ND], F32, isOutput=False)
    out = nc.declare_dram_parameter("hsum", [1, NWIN * ND], F32, isOutput=True)

    ALU = mybir.AluOpType
    ACT = mybir.ActivationFunctionType
    TB = 1024                    # columns per batched tile (2 chunks)
    NTILE = CQ // TB             # 20
    NSEG = 5
    SEGC = CQ // NSEG            # 4096 = 2 tiles
    MMN = 512                    # moving-dim per matmul (one PSUM bank span)

    with tile.TileContext(nc) as tc:
        with (
            tc.tile_pool(name="const", bufs=1) as cpool,
            tc.tile_pool(name="work", bufs=3) as pool,
            tc.tile_pool(name="big", bufs=1) as bigpool,
        ):
            # ---- preamble: constants ----
            w1_sb = cpool.tile([68, 128], BF16)
            nc.gpsimd.dma_start(w1_sb[:], w1[:])
            w2_sb = cpool.tile([128, 128], BF16)
            nc.gpsimd.dma_start(w2_sb[:], w2[:])
            w2b_sb = cpool.tile([68, 128], BF16)
            nc.gpsimd.dma_start(w2b_sb[:], w2b[:])
            b2_sb = cpool.tile([128, 1], F32)
            nc.gpsimd.dma_start(b2_sb[:], b2[:])

            # input segments: single-use tiles -> every DMA has at most one
            # wait (walrus DMA_DIRECT2D rejects multi-wait DMAs)
            ea_segs = []
            xt_segs = []
            for s in range(NSEG):
                ssl = slice(s * SEGC, (s + 1) * SEGC)
                et = bigpool.tile([68, SEGC], BF16, tag=f"ea{s}")
                nc.sync.dma_start(et[:], ea[:, ssl])
                ea_segs.append(et)
                xtt = bigpool.tile([128, SEGC], BF16, tag=f"xt{s}")
                nc.sync.dma_start(xtt[:], xt[:, ssl])
                xt_segs.append(xtt)

            oh_sb = cpool.tile([128, NT * 128], BF16)
            nc.gpsimd.dma_start(oh_sb[:], oh[:])
            xw_sb = cpool.tile([128, NWIN * ND], F32)
            nc.gpsimd.dma_start(xw_sb[:], xw[:])
            ident = cpool.tile([128, 128], F32)
            make_identity(nc, ident[:])
            ones_col = cpool.tile([128, 1], F32)
            nc.vector.memset(ones_col[:], 1.0)
            eps_col = cpool.tile([128, 1], F32)
            nc.vector.memset(eps_col[:], EPS)

            slotpart = bigpool.tile([128, S_CORE], F32)

            # ---- phase 1: per-edge pipeline, 10 tiles of 2048 columns ----
            with (
                tc.tile_pool(name="psA", bufs=2, space="PSUM") as psA,
                tc.tile_pool(name="psB", bufs=2, space="PSUM") as psB,
            ):
                for t in range(NTILE):
                    seg, off = t // 4, (t % 4) * TB
                    ea_sb = ea_segs[seg][:, off:off + TB]
                    xt_sb = xt_segs[seg][:, off:off + TB]

                    p1 = psA.tile([128, TB], F32, tag="p1")
                    for m in range(TB // MMN):
                        ms = slice(m * MMN, (m + 1) * MMN)
                        nc.tensor.matmul(p1[:, ms], lhsT=w1_sb[:],
                                         rhs=ea_sb[:, ms],
                                         start=True, stop=True)
                    # lrelu folded exactly: W2@lrelu(p1) = (.9W2)@relu(p1)
                    #                                     + (.1W2@W1)@ea
                    teh = pool.tile([128, TB], BF16, tag="teh")
                    nc.scalar.activation(teh[:], p1[:], ACT.Relu)

                    p2 = psB.tile([128, TB], F32, tag="p2")
                    for m in range(TB // MMN):
                        ms = slice(m * MMN, (m + 1) * MMN)
                        nc.tensor.matmul(p2[:, ms], lhsT=w2_sb[:],
                                         rhs=teh[:, ms],
                                         start=True, stop=False)
                        nc.tensor.matmul(p2[:, ms], lhsT=w2b_sb[:],
                                         rhs=ea_sb[:, ms],
                                         start=False, stop=True)
                    msg = pool.tile([128, TB], BF16, tag="msg")
                    nc.vector.tensor_tensor(
                        msg[:], p2[:], xt_sb, op=ALU.mult)
                    # 4 packed columns = 1 slot of 16 edges
                    nc.vector.tensor_reduce(
                        slotpart[:, t * (TB // 4):(t + 1) * (TB // 4)],
                        msg[:].rearrange("p (s x) -> p s x", x=4),
                        axis=mybir.AxisListType.X,
                        op=ALU.add,
                    )

            # ---- phase 2: slots -> nodes, LayerNorm, column sum ----
            # transpose [128=4gx32ch, 128slots] -> [128slots, 4gx32ch],
            # one-hot matmul to [128nodes, 4gx32ch], fold groups on free dim.
            with (
                tc.tile_pool(name="psT", bufs=2, space="PSUM") as psT,
                tc.tile_pool(name="psAgg", bufs=2, space="PSUM") as psAgg,
                tc.tile_pool(name="psHS", bufs=1, space="PSUM") as psHS,
            ):
                psm_all = bigpool.tile([128, NT * 128], BF16)
                for q in range(NT // 4):
                    pt = psT.tile([128, 512], F32, tag="pt")
                    for j in range(4):
                        k = 4 * q + j
                        nc.tensor.transpose(
                            pt[:, j * 128:(j + 1) * 128],
                            slotpart[:, k * 128:(k + 1) * 128], ident[:])
                    nc.scalar.copy(
                        psm_all[:, q * 512:(q + 1) * 512], pt[:])

                hs_psum = psHS.tile([1, NWIN * ND], F32)
                agg_sb = bigpool.tile([128, NWIN * 128], F32)
                for half in range(2):
                    aggp = psAgg.tile([128, 512], F32, tag="aggp")
                    for wi in range(4):
                        w = half * 4 + wi
                        for j in range(TPW):
                            k = w * TPW + j
                            nc.tensor.matmul(
                                aggp[:, wi * 128:(wi + 1) * 128],
                                lhsT=oh_sb[:, k * 128:(k + 1) * 128],
                                rhs=psm_all[:, k * 128:(k + 1) * 128],
                                start=(j == 0), stop=(j == TPW - 1))
                    nc.vector.tensor_copy(
                        agg_sb[:, half * 512:(half + 1) * 512], aggp[:])

                # fold 4 packed groups (free-dim slices of each window block)
                a_v = agg_sb[:].rearrange("p (w x) -> p w x", x=128)
                t2 = bigpool.tile([128, NWIN * 64], F32)
                t2_v = t2[:].rearrange("p (w x) -> p w x", x=64)
                nc.vector.tensor_tensor(
                    t2_v, a_v[:, :, 0:64], a_v[:, :, 64:128],
                    op=ALU.add)
                h_all = bigpool.tile([128, NWIN * ND], F32)
                h_v = h_all[:].rearrange("p (w x) -> p w x", x=ND)
                nc.vector.tensor_tensor(
                    h_v, t2_v[:, :, 0:ND], t2_v[:, :, ND:2 * ND],
                    op=ALU.add)
                nc.vector.tensor_add(h_all[:], h_all[:], xw_sb[:])

                # LayerNorm stats via reductions (batched over windows)
                hsum_w = bigpool.tile([128, NWIN], F32)
                nc.vector.tensor_reduce(
                    hsum_w[:], h_v, axis=mybir.AxisListType.X, op=ALU.add)
                hh = bigpool.tile([128, NWIN * ND], F32)
                nc.vector.tensor_mul(hh[:], h_all[:], h_all[:])
                hsq_w = bigpool.tile([128, NWIN], F32)
                nc.vector.tensor_reduce(
                    hsq_w[:], hh[:].rearrange("p (w x) -> p w x", x=ND),
                    axis=mybir.AxisListType.X, op=ALU.add)
                mu = bigpool.tile([128, NWIN], F32)
                nc.vector.tensor_scalar_mul(mu[:], hsum_w[:], 1.0 / ND)
                ex2 = bigpool.tile([128, NWIN], F32)
                nc.vector.tensor_scalar_mul(ex2[:], hsq_w[:], 1.0 / ND)
                mumu = bigpool.tile([128, NWIN], F32)
                nc.vector.tensor_mul(mumu[:], mu[:], mu[:])
                var = bigpool.tile([128, NWIN], F32)
                nc.vector.tensor_sub(var[:], ex2[:], mumu[:])
                sd = bigpool.tile([128, NWIN], F32)
                nc.scalar.activation(sd[:], var[:], ACT.Sqrt,
                                     bias=eps_col[:, 0:1])
                rc = bigpool.tile([128, NWIN], F32)
                nc.vector.reciprocal(rc[:], sd[:])

                z = bigpool.tile([128, NWIN * ND], F32)
                z_v = z[:].rearrange("p (w x) -> p w x", x=ND)
                mu_b = mu[:].rearrange("p (w x) -> p w x", x=1).to_broadcast(
                    [128, NWIN, ND])
                rc_b = rc[:].rearrange("p (w x) -> p w x", x=1).to_broadcast(
                    [128, NWIN, ND])
                nc.vector.tensor_tensor(z_v, h_v, mu_b, op=ALU.subtract)
                nc.vector.tensor_tensor(z_v, z_v, rc_b, op=ALU.mult)
                nc.tensor.matmul(hs_psum[:], lhsT=ones_col[:], rhs=z[:],
                                 start=True, stop=True)

                hs_sb = cpool.tile([1, NWIN * ND], F32)
                nc.vector.tensor_copy(hs_sb[:], hs_psum[:])
                nc.gpsimd.dma_start(out[:], hs_sb[:])
    nc.compile()
    return nc


def _get_program():
    if "nc" not in _PROG_CACHE:
        _PROG_CACHE["nc"] = _build_program()
    return _PROG_CACHE["nc"]


# --------------------------- host-side prep --------------------------------
def _prep_inputs(x, src, dst, edge_attr, W_e1, b_e1, W_e2, b_e2,
                 W_n1, b_n1, W_n2, b_n2, residual_weight, dot_scale):
    """Build the per-core input maps."""
    scale = np.float32(dot_scale.reshape(-1)[0] * residual_weight.reshape(-1)[0])

    # per-node transform x_t (tiny: 8192x32), exact fp32 as in reference
    x_t = _lrelu_np(x @ W_n1.T + b_n1) @ W_n2.T + b_n2
    x_t = x_t.astype(np.float32)

    # sort edges by destination
    perm = np.argsort(dst, kind="stable")
    dsts = dst[perm].astype(np.int64)
    srcs = src[perm].astype(np.int64)

    deg = np.bincount(dsts, minlength=N).astype(np.int64)
    cnt = (deg + SLOT - 1) // SLOT                  # slots per node
    padded = cnt * SLOT                              # padded edge count
    nstart = np.concatenate([[0], np.cumsum(deg)[:-1]])

    wid = np.arange(N) // 128                        # global window id (64)
    cum_p = np.cumsum(padded) - padded               # exclusive cumsum
    woff = cum_p[::128][wid]                         # window start in cum space
    pos_in_w = cum_p - woff
    if np.any(pos_in_w + padded > CAP_W):
        raise RuntimeError("window capacity exceeded; bump TPW and recompile")
    base_pos = wid * CAP_W + pos_in_w                # node -> first position

    NWTOT = NCORES * NWIN
    # edge j (sorted) -> position
    j = np.arange(E, dtype=np.int64)
    rank = j - nstart[dsts]
    pos = base_pos[dsts] + rank                      # in [0, NWTOT*CAP_W)

    sel = np.full(NWTOT * CAP_W, -1, dtype=np.int64)
    sel[pos] = j
    live = sel >= 0
    sel_c = np.where(live, sel, 0)

    ea_pos = edge_attr[perm][sel_c].astype(np.float32)
    ea_pos[~live] = 0.0
    xt_pos = x_t[srcs[sel_c]]
    xt_pos = np.where(live[:, None], xt_pos, np.float32(0.0)).astype(np.float32)
    ones_pos = live.astype(np.float32)

    # slot -> local node (for the level-2 one-hot), -1 for dead slots
    slotnode = np.full(NWTOT * SPW, -1, dtype=np.int64)
    tot_slots = int(cnt.sum())
    nodes_rep = np.repeat(np.arange(N), cnt)
    cum_slots = np.cumsum(cnt) - cnt
    ragged = np.arange(tot_slots) - np.repeat(cum_slots, cnt)
    slot_idx = np.repeat(base_pos // SLOT, cnt) + ragged
    slotnode[slot_idx] = nodes_rep

    # packed weights (shared across cores)
    w1p = np.zeros((68, 128), np.float32)
    w2p = np.zeros((128, 128), np.float32)
    w2bp = np.zeros((68, 128), np.float32)
    b2p = np.zeros((128, 1), np.float32)
    w2w1 = (W_e2 @ W_e1)            # [32, 16]
    w2b1 = (W_e2 @ b_e1)            # [32]
    for g in range(4):
        w1p[17 * g:17 * g + 16, 32 * g:32 * g + 32] = W_e1.T
        w1p[17 * g + 16, 32 * g:32 * g + 32] = b_e1
        w2p[32 * g:32 * g + 32, 32 * g:32 * g + 32] = W_e2.T * (0.9 * scale)
        w2bp[17 * g:17 * g + 16, 32 * g:32 * g + 32] = w2w1.T * (0.1 * scale)
        w2bp[17 * g + 16, 32 * g:32 * g + 32] = (
            w2b1 * 0.1 + b_e2) * scale
        b2p[32 * g:32 * g + 32, 0] = b_e2 * scale
    w1p = w1p.astype(BF)
    w2p = w2p.astype(BF)
    w2bp = w2bp.astype(BF)

    in_maps = []
    for c in range(NCORES):
        psl = slice(c * NWIN * CAP_W, (c + 1) * NWIN * CAP_W)
        ea_c = ea_pos[psl]                           # [EP, 16]
        on_c = ones_pos[psl]                         # [EP]
        xt_c = xt_pos[psl]                           # [EP, 32]

        # 4-pack: column q holds edge positions 4q+g for group g
        ea4 = ea_c.reshape(CQ, 4, ED).transpose(1, 2, 0)     # [4,16,CQ]
        on4 = on_c.reshape(CQ, 4).T                          # [4,CQ]
        ea4p = np.zeros((68, CQ), np.float32)
        for g in range(4):
            ea4p[17 * g:17 * g + 16] = ea4[g]
            ea4p[17 * g + 16] = on4[g]
        xt4p = np.ascontiguousarray(
            xt_c.reshape(CQ, 4, ND).transpose(1, 2, 0)).reshape(128, CQ)

        # one-hot [128 slots-in-tile, NT*128 node-cols]
        ssl = slice(c * S_CORE, (c + 1) * S_CORE)
        sn = slotnode[ssl]                                   # [S_CORE]
        ohc = np.zeros((128, NT * 128), np.float32)
        s_all = np.arange(S_CORE)
        livs = sn >= 0
        t_idx = s_all // 128
        r_idx = s_all % 128
        col = t_idx * 128 + (sn % 128)
        ohc[r_idx[livs], col[livs]] = 1.0

        xwc = np.ascontiguousarray(
            x[c * NPC:(c + 1) * NPC].reshape(NWIN, 128, ND)
            .transpose(1, 0, 2)).reshape(128, NWIN * ND).astype(np.float32)

        in_maps.append(dict(
            ea4p=ea4p.astype(BF), xt4p=xt4p.astype(BF), onehot=ohc.astype(BF),
            w1p=w1p, w2p=w2p, w2bp=w2bp, b2p=b2p, xw=xwc,
        ))
    return in_maps


# --------------------------------- entry -----------------------------------
def kernel(**inputs):
    global LAST_EXEC_NS, LAST_RESULTS
    x = np.asarray(inputs["x"], np.float32)
    ei = np.asarray(inputs["edge_index"])
    edge_attr = np.asarray(inputs["edge_attr"], np.float32)
    gamma = np.asarray(inputs["ln_gamma"], np.float32)
    beta = np.asarray(inputs["ln_beta"], np.float32)
    get = lambda k: np.asarray(inputs[k], np.float32)

    src = np.asarray(ei[0]).astype(np.int64)
    dst = np.asarray(ei[1]).astype(np.int64)

    in_maps = _prep_inputs(
        x, src, dst, edge_attr,
        get("W_e1"), get("b_e1"), get("W_e2"), get("b_e2"),
        get("W_n1"), get("b_n1"), get("W_n2"), get("b_n2"),
        get("residual_weight"), get("dot_scale"))

    nc = _get_program()
    trace = bool(int(os.environ.get("BASS_GNN_TRACE", "0")))
    res = run_bass_kernel_spmd(nc, in_maps, list(range(NCORES)), trace=trace)
    LAST_RESULTS = res
    LAST_EXEC_NS = getattr(res, "exec_time_ns", None)

    hs = np.zeros(ND, np.float64)
    for r in res.results:
        hs += np.asarray(r["hsum"], np.float64).reshape(NWIN, ND).sum(0)
    hs = hs.astype(np.float32)

    new_x_row = (gamma * hs + np.float32(N) * beta) / np.float32(K_CLUST)
    new_x = np.broadcast_to(new_x_row.astype(np.float32), (K_CLUST, ND)).copy()
    idx_dtype = ei.dtype if np.issubdtype(ei.dtype, np.integer) else np.int32
    new_edge_index = np.zeros((2, E), idx_dtype)
    new_edge_attr = np.zeros((E, ED), np.float32)
    A = np.full((N, K_CLUST), np.float32(1.0 / K_CLUST), np.float32)
    new_node_types = np.zeros((K_CLUST, 1), np.float32)
    edge_mask = np.zeros((E,), bool)
    return (new_x, new_edge_index, new_edge_attr, A, new_node_types, edge_mask)
